# revision 41
# baseline (speedup 1.0000x reference)
"""NeuS volume-rendering kernel for 8 Trainium2 NeuronCores.

Math: with sig = sigmoid(s*sdf), the NeuS cumprod telescopes:
  1 - alpha[k] = sig[k+1]/sig[k]  =>  trans[i] = sig[i]/sig[0]
  weight[i] = relu(sig[i] - sig[i+1]) / sig[0]   (i = 1..S-2; w[0] = w[S-1] = 0)
  pixel[c]  = sum_s w*color_c + (1 - sum_s w)*bg_c
  invdepth  = sum_s w / z

Sharded data-parallel over rays across 8 cores; everything per-ray is local.

Per-ray sums use a fused custom DVE op: scan(ADD, Src0*Src1) with a step-0
output access pattern — every element of a 128-sample page writes the running
prefix to the same address, so the surviving value is the cumulative sum at
page end. Per-page sums are recovered with one shifted subtract at the end.
"""

import threading

import numpy as np

R_TOTAL = 65536
S = 128
N_CORES = 8
R_CORE = R_TOTAL // N_CORES  # 8192
P = 128  # rays per tile (partition dim)
TPB = 16  # tiles per DMA block
HB = 8  # tiles per compute half-block


def _register_scan_ops():
    """Fused multiply+running-sum custom DVE ops."""
    import concourse.dve_ops as dops
    from concourse.dve_spec import C0, AluOp, Spec, Src0, Src1, scan
    from concourse.dve_spec import lower as dve_lower
    from concourse.dve_uop import DveOpSpec

    def make(name, body, ref, rd1):
        if name in dops._SUB_OPCODE_FOR_NAME:
            return next(o for o in dops.OPS if o.name == name)
        spec = Spec(body=body, reference=ref)
        opcode = dops._CUSTOM_DVE_ROW_BASE + len(dops.OPS)
        shas = {}
        for ver in ("v3", "v4"):
            sp = DveOpSpec(
                name=name, opcode=opcode, uops=dve_lower(spec, ver=ver), rd1_en=rd1
            )
            shas[ver] = sp.sha(ver)
        op = dops.DveOp(name, spec, subdim=False, uops_sha=shas)
        dops.OPS.append(op)
        dops.CUSTOM_DVE_SPECS[name] = spec
        dops._SUB_OPCODE_FOR_NAME[name] = opcode
        return op

    def ref_mul(in0, in1, c0, c1, c2):
        p = (in0.astype(np.float32) * in1.astype(np.float32)).reshape(
            in0.shape[0], -1
        )
        return np.cumsum(p, axis=1, dtype=np.float32).reshape(in0.shape)

    def ref_msub(in0, in1, c0, c1, c2):
        c0 = np.asarray(c0, np.float32).reshape((-1,) + (1,) * (in0.ndim - 1))
        p = ((in0.astype(np.float32) - c0) * in1.astype(np.float32)).reshape(
            in0.shape[0], -1
        )
        return np.cumsum(p, axis=1, dtype=np.float32).reshape(in0.shape)

    mul_scan = make("MUL_SCAN_ANT", scan(AluOp.ADD, Src0 * Src1), ref_mul, True)
    msub_scan = make(
        "MSUB_SCAN_ANT", scan(AluOp.ADD, (Src0 - C0) * Src1), ref_msub, True
    )
    return mul_scan, msub_scan


def _build(r_core: int):
    from contextlib import ExitStack

    import concourse.bacc as bacc
    import concourse.mybir as mybir
    import concourse.tile as tile

    f32 = mybir.dt.float32
    AF = mybir.ActivationFunctionType
    ALU = mybir.AluOpType

    MUL_SCAN, MSUB_SCAN = _register_scan_ops()

    T = r_core // P  # tiles per core
    NST = T // TPB  # super-tiles per core

    nc = bacc.Bacc("TRN2", target_bir_lowering=False, debug=False)

    sdf_d = nc.dram_tensor("sdf", [r_core, S], f32, kind="ExternalInput").ap()
    col_d = nc.dram_tensor("color", [r_core, S * 3], f32, kind="ExternalInput").ap()
    z_d = nc.dram_tensor("z_vals", [r_core, S], f32, kind="ExternalInput").ap()
    s_d = nc.dram_tensor("s", [1], f32, kind="ExternalInput").ap()
    bg_d = nc.dram_tensor("bg_color", [3], f32, kind="ExternalInput").ap()

    w_d = nc.dram_tensor("weight", [r_core, S], f32, kind="ExternalOutput").ap()
    pix_d = nc.dram_tensor("pixel", [r_core, 3], f32, kind="ExternalOutput").ap()
    inv_d = nc.dram_tensor("invdepth", [r_core], f32, kind="ExternalOutput").ap()

    # super-tile views: ray = p*T + st*TPB + t (partition-major so each
    # partition's rays are contiguous in DRAM -> large DMA descriptors)
    sdf_v = sdf_d.rearrange("(p n t) s -> n p t s", t=TPB, p=P)
    z_v = z_d.rearrange("(p n t) s -> n p t s", t=TPB, p=P)
    col_v = col_d.rearrange("(p n t) c -> n p t c", t=TPB, p=P)
    w_v = w_d.rearrange("(p n t) s -> n p t s", t=TPB, p=P)
    pix_v = pix_d.rearrange("(p t) c -> p t c", p=P)  # [128, T, 3]
    inv_v = inv_d.rearrange("(p t) -> p t", p=P)  # [128, T]

    with tile.TileContext(nc) as tc, ExitStack() as ctx:
        const_pool = ctx.enter_context(tc.tile_pool(name="const", bufs=1))
        acc_pool = ctx.enter_context(tc.tile_pool(name="acc", bufs=1))
        in_pool = ctx.enter_context(tc.tile_pool(name="in", bufs=4))
        col_pool = ctx.enter_context(tc.tile_pool(name="colp", bufs=2))
        mid_pool = ctx.enter_context(tc.tile_pool(name="mid", bufs=2))
        w_pool = ctx.enter_context(tc.tile_pool(name="wp", bufs=3))

        # ---- constants: s, bg broadcast to all partitions (step-0 DMA) ----
        s_bc = const_pool.tile([P, 1], f32, tag="sbc")
        nc.gpsimd.dma_start(s_bc[:], s_d[None, :].partition_broadcast(P))

        bg_bc = const_pool.tile([P, 3], f32, tag="bgbc")
        nc.gpsimd.dma_start(bg_bc[:], bg_d[None, :].partition_broadcast(P))

        # ---- accumulators: raw scan cums + fixed per-tile sums ----
        cum_acc = acc_pool.tile([P, T, 4], f32, tag="cumacc")  # r,g,b,invd
        acc_fix = acc_pool.tile([P, T, 4], f32, tag="accfix")

        rs_all = const_pool.tile([P, T], f32, tag="rs")
        pix_fin = const_pool.tile([P, T, 3], f32, tag="pixfin")

        # ---- main loop over super-tiles ----
        for st in range(NST):
            sdf_t = in_pool.tile([P, TPB, S], f32, tag="sdf")
            z_t = in_pool.tile([P, TPB, S], f32, tag="z")
            col_t = col_pool.tile([P, TPB, S * 3], f32, tag="col")
            if st == 0:
                for h in range(TPB // HB):
                    hs = slice(h * HB, (h + 1) * HB)
                    nc.sync.dma_start(sdf_t[:, hs], sdf_v[st][:, hs])
                    nc.scalar.dma_start(col_t[:, hs], col_v[st][:, hs])
                    nc.sync.dma_start(z_t[:, hs], z_v[st][:, hs])
            else:
                nc.sync.dma_start(sdf_t[:], sdf_v[st])
                nc.sync.dma_start(z_t[:], z_v[st])
                nc.scalar.dma_start(col_t[:], col_v[st])

            for h in range(TPB // HB):
                hsl = slice(h * HB, (h + 1) * HB)
                lo = st * TPB + h * HB

                sig_t = mid_pool.tile([P, HB, S], f32, tag="sig")
                nc.scalar.activation(
                    sig_t[:].rearrange("p t s -> p (t s)"),
                    sdf_t[:, hsl].rearrange("p t s -> p (t s)"),
                    AF.Sigmoid,
                    scale=s_bc[:],
                )

                # rs = 1 / sig[:, 0] for this half-block's rays
                nc.vector.reciprocal(rs_all[:, lo : lo + HB], sig_t[:, :, 0])

                rz_t = mid_pool.tile([P, HB, S], f32, tag="rz")
                nc.vector.reciprocal_approx_fast(
                    rz_t[:].rearrange("p t s -> p (t s)"),
                    z_t[:, hsl].rearrange("p t s -> p (t s)"),
                )

                # d[i] = sig[i] - sig[i+1] for i=1..126  (on GPSIMD)
                d_t = mid_pool.tile([P, HB, S - 2], f32, tag="d")
                nc.gpsimd.tensor_sub(
                    d_t[:], sig_t[:, :, 1 : S - 1], sig_t[:, :, 2:S]
                )

                if h == 0:
                    w_b = w_pool.tile([P, TPB, S], f32, tag="w")
                w_t = w_b[:, hsl]
                nc.gpsimd.memset(w_t[:, :, 0:1], 0.0)
                nc.gpsimd.memset(w_t[:, :, S - 1 : S], 0.0)

                for i in range(HB):
                    # w = relu(d * rs) on ACT (sigmoid/relu share a table set)
                    nc.scalar.activation(
                        w_t[:, i, 1 : S - 1],
                        d_t[:, i, :],
                        AF.Relu,
                        scale=rs_all[:, lo + i : lo + i + 1],
                    )

                # fused (color-bg)*w product + running-sum; page j's cum lands
                # at out addr j (step-0 inner out AP)
                for c in range(3):
                    nc.vector._custom_dve(
                        MSUB_SCAN,
                        out=cum_acc[:, lo : lo + HB, c].unsqueeze(2).to_broadcast(
                            [P, HB, S]
                        ),
                        in0=col_t[:, hsl].rearrange("p t (s c) -> p t s c", c=3)[
                            :, :, :, c
                        ],
                        in1=w_t[:],
                        s0=bg_bc[:, c : c + 1],
                    )
                nc.vector._custom_dve(
                    MUL_SCAN,
                    out=cum_acc[:, lo : lo + HB, 3].unsqueeze(2).to_broadcast(
                        [P, HB, S]
                    ),
                    in0=rz_t[:],
                    in1=w_t[:],
                )

                if h == TPB // HB - 1:
                    nc.gpsimd.dma_start(w_v[st], w_b[:])

        # ---- fixup: per-page sums = cum[j] - cum[j-1] within each scan ----
        cv = cum_acc[:].rearrange("p (n t) c -> p n t c", t=HB)
        av = acc_fix[:].rearrange("p (n t) c -> p n t c", t=HB)
        nc.vector.tensor_sub(av[:, :, 1:HB, :], cv[:, :, 1:HB, :], cv[:, :, 0 : HB - 1, :])
        nc.vector.tensor_copy(av[:, :, 0, :], cv[:, :, 0, :])

        # ---- pixel = bg + sum(w*(c-bg))  (bg folded into the scan) ----
        for c in range(3):
            nc.vector.tensor_scalar_add(
                pix_fin[:, :, c], acc_fix[:, :, c], bg_bc[:, c : c + 1]
            )

        inv_c = const_pool.tile([P, T], f32, tag="invc")
        nc.vector.tensor_copy(inv_c[:], acc_fix[:, :, 3])
        nc.sync.dma_start(pix_v, pix_fin[:])
        nc.sync.dma_start(inv_v, inv_c[:])

    nc.compile()
    return nc


_lock = threading.Lock()
_cache: dict = {}


def _get_nc(r_core: int):
    with _lock:
        if r_core not in _cache:
            _cache[r_core] = _build(r_core)
        return _cache[r_core]


def kernel(sdf, color, z_vals, s, bg_color):
    from concourse.bass_utils import run_bass_kernel_spmd

    sdf = np.ascontiguousarray(sdf, dtype=np.float32)
    color = np.ascontiguousarray(color, dtype=np.float32)
    z_vals = np.ascontiguousarray(z_vals, dtype=np.float32)
    s = np.ascontiguousarray(s, dtype=np.float32)
    bg_color = np.ascontiguousarray(bg_color, dtype=np.float32)

    nc = _get_nc(R_CORE)

    in_maps = []
    for k in range(N_CORES):
        lo, hi = k * R_CORE, (k + 1) * R_CORE
        in_maps.append(
            {
                "sdf": sdf[lo:hi],
                "color": color[lo:hi].reshape(R_CORE, S * 3),
                "z_vals": z_vals[lo:hi],
                "s": s,
                "bg_color": bg_color,
            }
        )

    res = run_bass_kernel_spmd(nc, in_maps, core_ids=list(range(N_CORES)))
    outs = res.results

    pixel = np.concatenate([r["pixel"] for r in outs], axis=0)
    invdepth = np.concatenate([r["invdepth"] for r in outs], axis=0)
    weight = np.concatenate([r["weight"] for r in outs], axis=0)
    return pixel, invdepth, weight


# revision 42
# speedup vs baseline: 1.0292x; 1.0292x over previous
"""NeuS volume-rendering kernel for 8 Trainium2 NeuronCores.

Math: with sig = sigmoid(s*sdf), the NeuS cumprod telescopes:
  1 - alpha[k] = sig[k+1]/sig[k]  =>  trans[i] = sig[i]/sig[0]
  weight[i] = relu(sig[i] - sig[i+1]) / sig[0]   (i = 1..S-2; w[0] = w[S-1] = 0)
  pixel[c]  = sum_s w*color_c + (1 - sum_s w)*bg_c
  invdepth  = sum_s w / z

Sharded data-parallel over rays across 8 cores; everything per-ray is local.

Per-ray sums use a fused custom DVE op: scan(ADD, Src0*Src1) with a step-0
output access pattern — every element of a 128-sample page writes the running
prefix to the same address, so the surviving value is the cumulative sum at
page end. Per-page sums are recovered with one shifted subtract at the end.
"""

import threading

import numpy as np

R_TOTAL = 65536
S = 128
N_CORES = 8
R_CORE = R_TOTAL // N_CORES  # 8192
P = 128  # rays per tile (partition dim)
TPB = 16  # tiles per DMA block
HB = 8  # tiles per compute half-block


def _register_scan_ops():
    """Fused multiply+running-sum custom DVE ops."""
    import concourse.dve_ops as dops
    from concourse.dve_spec import C0, AluOp, Spec, Src0, Src1, scan
    from concourse.dve_spec import lower as dve_lower
    from concourse.dve_uop import DveOpSpec

    def make(name, body, ref, rd1):
        if name in dops._SUB_OPCODE_FOR_NAME:
            return next(o for o in dops.OPS if o.name == name)
        spec = Spec(body=body, reference=ref)
        opcode = dops._CUSTOM_DVE_ROW_BASE + len(dops.OPS)
        shas = {}
        for ver in ("v3", "v4"):
            sp = DveOpSpec(
                name=name, opcode=opcode, uops=dve_lower(spec, ver=ver), rd1_en=rd1
            )
            shas[ver] = sp.sha(ver)
        op = dops.DveOp(name, spec, subdim=False, uops_sha=shas)
        dops.OPS.append(op)
        dops.CUSTOM_DVE_SPECS[name] = spec
        dops._SUB_OPCODE_FOR_NAME[name] = opcode
        return op

    def ref_mul(in0, in1, c0, c1, c2):
        p = (in0.astype(np.float32) * in1.astype(np.float32)).reshape(
            in0.shape[0], -1
        )
        return np.cumsum(p, axis=1, dtype=np.float32).reshape(in0.shape)

    def ref_msub(in0, in1, c0, c1, c2):
        c0 = np.asarray(c0, np.float32).reshape((-1,) + (1,) * (in0.ndim - 1))
        p = ((in0.astype(np.float32) - c0) * in1.astype(np.float32)).reshape(
            in0.shape[0], -1
        )
        return np.cumsum(p, axis=1, dtype=np.float32).reshape(in0.shape)

    mul_scan = make("MUL_SCAN_ANT", scan(AluOp.ADD, Src0 * Src1), ref_mul, True)
    msub_scan = make(
        "MSUB_SCAN_ANT", scan(AluOp.ADD, (Src0 - C0) * Src1), ref_msub, True
    )
    return mul_scan, msub_scan


def _build(r_core: int):
    from contextlib import ExitStack

    import concourse.bacc as bacc
    import concourse.mybir as mybir
    import concourse.tile as tile

    f32 = mybir.dt.float32
    AF = mybir.ActivationFunctionType
    ALU = mybir.AluOpType

    MUL_SCAN, MSUB_SCAN = _register_scan_ops()

    T = r_core // P  # tiles per core
    NST = T // TPB  # super-tiles per core

    nc = bacc.Bacc("TRN2", target_bir_lowering=False, debug=False)

    sdf_d = nc.dram_tensor("sdf", [r_core, S], f32, kind="ExternalInput").ap()
    col_d = nc.dram_tensor("color", [r_core, S * 3], f32, kind="ExternalInput").ap()
    z_d = nc.dram_tensor("z_vals", [r_core, S], f32, kind="ExternalInput").ap()
    s_d = nc.dram_tensor("s", [1], f32, kind="ExternalInput").ap()
    bg_d = nc.dram_tensor("bg_color", [3], f32, kind="ExternalInput").ap()

    w_d = nc.dram_tensor("weight", [r_core, S], f32, kind="ExternalOutput").ap()
    pix_d = nc.dram_tensor("pixel", [r_core, 3], f32, kind="ExternalOutput").ap()
    inv_d = nc.dram_tensor("invdepth", [r_core], f32, kind="ExternalOutput").ap()

    # super-tile views: ray = p*T + st*TPB + t (partition-major so each
    # partition's rays are contiguous in DRAM -> large DMA descriptors)
    sdf_v = sdf_d.rearrange("(p n t) s -> n p t s", t=TPB, p=P)
    z_v = z_d.rearrange("(p n t) s -> n p t s", t=TPB, p=P)
    col_v = col_d.rearrange("(p n t) c -> n p t c", t=TPB, p=P)
    w_v = w_d.rearrange("(p n t) s -> n p t s", t=TPB, p=P)
    pix_v = pix_d.rearrange("(p t) c -> p t c", p=P)  # [128, T, 3]
    inv_v = inv_d.rearrange("(p t) -> p t", p=P)  # [128, T]

    with tile.TileContext(nc) as tc, ExitStack() as ctx:
        const_pool = ctx.enter_context(tc.tile_pool(name="const", bufs=1))
        acc_pool = ctx.enter_context(tc.tile_pool(name="acc", bufs=1))
        in_pool = ctx.enter_context(tc.tile_pool(name="in", bufs=4))
        col_pool = ctx.enter_context(tc.tile_pool(name="colp", bufs=2))
        mid_pool = ctx.enter_context(tc.tile_pool(name="mid", bufs=2))
        w_pool = ctx.enter_context(tc.tile_pool(name="wp", bufs=3))

        # ---- constants: s, bg broadcast to all partitions (step-0 DMA) ----
        s_bc = const_pool.tile([P, 1], f32, tag="sbc")
        nc.sync.dma_start(s_bc[:], s_d[None, :].partition_broadcast(P))

        bg_bc = const_pool.tile([P, 3], f32, tag="bgbc")
        nc.sync.dma_start(bg_bc[:], bg_d[None, :].partition_broadcast(P))

        # ---- accumulators: raw scan cums + fixed per-tile sums ----
        cum_acc = acc_pool.tile([P, T, 4], f32, tag="cumacc")  # r,g,b,invd
        acc_fix = acc_pool.tile([P, T, 4], f32, tag="accfix")

        rs_all = const_pool.tile([P, T], f32, tag="rs")
        pix_fin = const_pool.tile([P, T, 3], f32, tag="pixfin")

        # ---- main loop over super-tiles ----
        for st in range(NST):
            sdf_t = in_pool.tile([P, TPB, S], f32, tag="sdf")
            nc.sync.dma_start(sdf_t[:], sdf_v[st])
            z_t = in_pool.tile([P, TPB, S], f32, tag="z")
            nc.sync.dma_start(z_t[:], z_v[st])
            col_t = col_pool.tile([P, TPB, S * 3], f32, tag="col")
            nc.scalar.dma_start(col_t[:], col_v[st])

            for h in range(TPB // HB):
                hsl = slice(h * HB, (h + 1) * HB)
                lo = st * TPB + h * HB

                sig_t = mid_pool.tile([P, HB, S], f32, tag="sig")
                nc.scalar.activation(
                    sig_t[:].rearrange("p t s -> p (t s)"),
                    sdf_t[:, hsl].rearrange("p t s -> p (t s)"),
                    AF.Sigmoid,
                    scale=s_bc[:],
                )

                # rs = 1 / sig[:, 0] for this half-block's rays
                nc.vector.reciprocal(rs_all[:, lo : lo + HB], sig_t[:, :, 0])

                rz_t = mid_pool.tile([P, HB, S], f32, tag="rz")
                nc.vector.reciprocal_approx_fast(
                    rz_t[:].rearrange("p t s -> p (t s)"),
                    z_t[:, hsl].rearrange("p t s -> p (t s)"),
                )

                # d[i] = sig[i] - sig[i+1] for i=1..126  (on GPSIMD)
                d_t = mid_pool.tile([P, HB, S - 2], f32, tag="d")
                nc.gpsimd.tensor_sub(
                    d_t[:], sig_t[:, :, 1 : S - 1], sig_t[:, :, 2:S]
                )

                w_t = w_pool.tile([P, HB, S], f32, tag="w")
                nc.gpsimd.memset(w_t[:, :, 0:1], 0.0)
                nc.gpsimd.memset(w_t[:, :, S - 1 : S], 0.0)

                for i in range(HB):
                    # w = relu(d * rs) on ACT (sigmoid/relu share a table set)
                    nc.scalar.activation(
                        w_t[:, i, 1 : S - 1],
                        d_t[:, i, :],
                        AF.Relu,
                        scale=rs_all[:, lo + i : lo + i + 1],
                    )

                # fused (color-bg)*w product + running-sum; page j's cum lands
                # at out addr j (step-0 inner out AP)
                for c in range(3):
                    nc.vector._custom_dve(
                        MSUB_SCAN,
                        out=cum_acc[:, lo : lo + HB, c].unsqueeze(2).to_broadcast(
                            [P, HB, S]
                        ),
                        in0=col_t[:, hsl].rearrange("p t (s c) -> p t s c", c=3)[
                            :, :, :, c
                        ],
                        in1=w_t[:],
                        s0=bg_bc[:, c : c + 1],
                    )
                nc.vector._custom_dve(
                    MUL_SCAN,
                    out=cum_acc[:, lo : lo + HB, 3].unsqueeze(2).to_broadcast(
                        [P, HB, S]
                    ),
                    in0=rz_t[:],
                    in1=w_t[:],
                )

                nc.gpsimd.dma_start(w_v[st][:, hsl], w_t[:])

        # ---- fixup: per-page sums = cum[j] - cum[j-1] within each scan ----
        cv = cum_acc[:].rearrange("p (n t) c -> p n t c", t=HB)
        av = acc_fix[:].rearrange("p (n t) c -> p n t c", t=HB)
        nc.vector.tensor_sub(av[:, :, 1:HB, :], cv[:, :, 1:HB, :], cv[:, :, 0 : HB - 1, :])
        nc.vector.tensor_copy(av[:, :, 0, :], cv[:, :, 0, :])

        # ---- pixel = bg + sum(w*(c-bg))  (bg folded into the scan) ----
        for c in range(3):
            nc.vector.tensor_scalar_add(
                pix_fin[:, :, c], acc_fix[:, :, c], bg_bc[:, c : c + 1]
            )

        inv_c = const_pool.tile([P, T], f32, tag="invc")
        nc.vector.tensor_copy(inv_c[:], acc_fix[:, :, 3])
        nc.sync.dma_start(pix_v, pix_fin[:])
        nc.sync.dma_start(inv_v, inv_c[:])

    nc.compile()
    return nc


_lock = threading.Lock()
_cache: dict = {}


def _get_nc(r_core: int):
    with _lock:
        if r_core not in _cache:
            _cache[r_core] = _build(r_core)
        return _cache[r_core]


def kernel(sdf, color, z_vals, s, bg_color):
    from concourse.bass_utils import run_bass_kernel_spmd

    sdf = np.ascontiguousarray(sdf, dtype=np.float32)
    color = np.ascontiguousarray(color, dtype=np.float32)
    z_vals = np.ascontiguousarray(z_vals, dtype=np.float32)
    s = np.ascontiguousarray(s, dtype=np.float32)
    bg_color = np.ascontiguousarray(bg_color, dtype=np.float32)

    nc = _get_nc(R_CORE)

    in_maps = []
    for k in range(N_CORES):
        lo, hi = k * R_CORE, (k + 1) * R_CORE
        in_maps.append(
            {
                "sdf": sdf[lo:hi],
                "color": color[lo:hi].reshape(R_CORE, S * 3),
                "z_vals": z_vals[lo:hi],
                "s": s,
                "bg_color": bg_color,
            }
        )

    res = run_bass_kernel_spmd(nc, in_maps, core_ids=list(range(N_CORES)))
    outs = res.results

    pixel = np.concatenate([r["pixel"] for r in outs], axis=0)
    invdepth = np.concatenate([r["invdepth"] for r in outs], axis=0)
    weight = np.concatenate([r["weight"] for r in outs], axis=0)
    return pixel, invdepth, weight
